# revision 7
# baseline (speedup 1.0000x reference)
"""Trainium2 Bass kernel for nn_ComponentWiseSpline (rational-quadratic spline fwd).

Self-contained: hardcodes shapes B=65536, D=128, K=8, 8-core batch-data-parallel.

Algorithm (d-on-partitions layout, per core shard of B/8 rows):
  Host (numpy, f64->f32): spline tables -> 9 per-(d,bin) "plane" tables,
  each encoded as base + 7 telescoped deltas over step masks
  m_k = (xc >= knot_k + eps).  Device:
    xc   = clip(x, -3, 3)
    m_k  = (xc >= thr_k)                        k=1..7
    P_q  = base_q + sum_k m_k * delta_qk        (9 gathered planes)
    u    = xc - P_c;  th = u * P_rw;  th2 = th^2;  t1m = th - th2
    num* = P_ih*th2 + P_P1*t1m                  (= num/delta)
    den* = 1 + P_Rs*t1m                         (= den/delta)
    dnum'= P_d1g*th2 + P_dd*t1m + P_d0g*(th-1)^2 (= dnum/delta^2, positive form)
    out  = P_ich + num*/den*;   lad = ln(dnum') - 2 ln(den*)
    identity outside |x|<=3;  log_detJ = ones^T @ lad (PE partition-reduce)
"""
import os
import sys

for _p in ("/opt/trn_rl_repo", os.path.expanduser("~/.axon_site/_ro/trn_rl_repo")):
    if os.path.isdir(_p) and _p not in sys.path:
        sys.path.insert(0, _p)

import numpy as np

B, D, K = 65536, 128, 8
N_CORES = 8
B_SHARD = B // N_CORES
BOUND = 3.0
MIN_BIN_W = 1e-3
MIN_BIN_H = 1e-3
MIN_DERIV = 1e-3
EPS = 1e-6

PLANES = ["c", "rw", "ih", "P1", "Rs", "d1g", "dd", "d0g", "ich"]
NTAB = 7 + 8 * len(PLANES)  # 7 thresholds + (base + 7 deltas) per plane


# --------------------------------------------------------------------------- #
# Host-side table construction (numpy, f64 internally)
# --------------------------------------------------------------------------- #
def _make_tab(uw, uh, ud):
    uw = np.asarray(uw, np.float64)
    uh = np.asarray(uh, np.float64)
    ud = np.asarray(ud, np.float64)

    def softmax(v):
        e = np.exp(v - v.max(axis=-1, keepdims=True))
        return e / e.sum(axis=-1, keepdims=True)

    def softplus(v):
        return np.log1p(np.exp(-np.abs(v))) + np.maximum(v, 0)

    widths = MIN_BIN_W + (1.0 - MIN_BIN_W * K) * softmax(uw)
    heights = MIN_BIN_H + (1.0 - MIN_BIN_H * K) * softmax(uh)
    derivs = np.pad(MIN_DERIV + softplus(ud), ((0, 0), (1, 1)),
                    constant_values=1.0 - MIN_DERIV)  # [D, K+1]

    def knots(lengths):
        kn = np.cumsum(lengths, axis=-1)
        kn = np.pad(kn, ((0, 0), (1, 0)))
        kn = 2 * BOUND * kn - BOUND
        kn[:, 0] = -BOUND
        kn[:, -1] = BOUND
        return kn[:, 1:] - kn[:, :-1], kn

    w_, cw = knots(widths)
    h_, ch = knots(heights)
    delta = h_ / w_
    d0 = derivs[:, :K]
    d1 = derivs[:, 1:]

    tabs = {
        "c": cw[:, :K],
        "rw": 1.0 / w_,
        "ih": h_,
        "P1": h_ * d0 / delta,
        "Rs": (d0 + d1 - 2 * delta) / delta,
        "d1g": d1,
        "dd": 2 * delta,
        "d0g": d0,
        "ich": ch[:, :K],
    }

    tab = np.zeros((D, NTAB), np.float32)
    tab[:, 0:7] = (cw[:, 1:K] + EPS).astype(np.float32)
    for q, name in enumerate(PLANES):
        t = tabs[name].astype(np.float32).astype(np.float64)
        base = 7 + 8 * q
        tab[:, base] = t[:, 0].astype(np.float32)
        tab[:, base + 1:base + 8] = (t[:, 1:] - t[:, :-1]).astype(np.float32)
    return tab


# --------------------------------------------------------------------------- #
# Device kernel
# --------------------------------------------------------------------------- #
def build_nc(b_shard=B_SHARD, fd=512):
    import concourse.bacc as bacc
    import concourse.mybir as mybir
    from concourse.tile import TileContext
    from contextlib import ExitStack

    f32 = mybir.dt.float32
    op = mybir.AluOpType
    AF = mybir.ActivationFunctionType

    assert b_shard % fd == 0
    nchunks = b_shard // fd

    nc = bacc.Bacc("TRN2", target_bir_lowering=False, debug=False)
    x = nc.dram_tensor("x", [b_shard, D], f32, kind="ExternalInput")
    tabd = nc.dram_tensor("tab", [D, NTAB], f32, kind="ExternalInput")
    u_out = nc.dram_tensor("u", [b_shard, D], f32, kind="ExternalOutput")
    ld_out = nc.dram_tensor("ld", [1, b_shard], f32, kind="ExternalOutput")

    def col(tile, j):  # [128, 1] per-partition scalar view
        return tile[:, j:j + 1]

    with TileContext(nc) as tc, ExitStack() as ctx:
        cpool = ctx.enter_context(tc.tile_pool(name="const", bufs=1))
        iopool = ctx.enter_context(tc.tile_pool(name="io", bufs=2))
        pool = ctx.enter_context(tc.tile_pool(name="work", bufs=1))
        ppool = ctx.enter_context(tc.tile_pool(name="psum", bufs=2, space="PSUM"))

        tab = cpool.tile([D, NTAB], f32, tag="tab")
        nc.sync.dma_start(tab[:], tabd[:])
        ones = cpool.tile([D, 1], f32, tag="ones")
        nc.vector.memset(ones[:], 1.0)

        for ci in range(nchunks):
            cs = ci * fd
            xT = iopool.tile([D, fd], f32, tag="xT")
            nc.sync.dma_start(xT[:], x[cs:cs + fd, :].rearrange("a b -> b a"))

            xc = pool.tile([D, fd], f32, tag="xc")
            nc.vector.tensor_scalar(xc[:], xT[:], -BOUND, BOUND, op.max, op.min)

            masks = []
            for k in range(1, 8):
                m = pool.tile([D, fd], f32, tag=f"m{k}")
                nc.vector.tensor_scalar(m[:], xc[:], col(tab, k - 1), None, op.is_ge)
                masks.append(m)

            planes = {}
            for q, name in enumerate(PLANES):
                base = 7 + 8 * q
                p = pool.tile([D, fd], f32, tag=f"p_{name}")
                nc.vector.tensor_scalar(p[:], masks[0][:], col(tab, base + 1),
                                        col(tab, base), op.mult, op.add)
                for k in range(2, 8):
                    nc.vector.scalar_tensor_tensor(p[:], masks[k - 1][:],
                                                   col(tab, base + k), p[:],
                                                   op.mult, op.add)
                planes[name] = p

            t1 = pool.tile([D, fd], f32, tag="t1")   # u, then th
            nc.vector.tensor_tensor(t1[:], xc[:], planes["c"][:], op.subtract)
            nc.vector.tensor_tensor(t1[:], t1[:], planes["rw"][:], op.mult)  # th
            th2 = pool.tile([D, fd], f32, tag="th2")
            nc.vector.tensor_tensor(th2[:], t1[:], t1[:], op.mult)
            t1m = pool.tile([D, fd], f32, tag="t1m")
            nc.vector.tensor_tensor(t1m[:], t1[:], th2[:], op.subtract)
            m1t = pool.tile([D, fd], f32, tag="m1t")  # th-1, then (th-1)^2
            nc.vector.tensor_scalar(m1t[:], t1[:], 1.0, None, op.subtract)
            nc.vector.tensor_tensor(m1t[:], m1t[:], m1t[:], op.mult)

            num = pool.tile([D, fd], f32, tag="num")
            tmp = pool.tile([D, fd], f32, tag="tmp")
            nc.vector.tensor_tensor(num[:], planes["ih"][:], th2[:], op.mult)
            nc.vector.tensor_tensor(tmp[:], planes["P1"][:], t1m[:], op.mult)
            nc.vector.tensor_tensor(num[:], num[:], tmp[:], op.add)

            den = pool.tile([D, fd], f32, tag="den")
            nc.vector.tensor_tensor(den[:], planes["Rs"][:], t1m[:], op.mult)
            nc.vector.tensor_scalar(den[:], den[:], 1.0, None, op.add)

            dnum = pool.tile([D, fd], f32, tag="dnum")
            nc.vector.tensor_tensor(dnum[:], planes["d1g"][:], th2[:], op.mult)
            nc.vector.tensor_tensor(tmp[:], planes["dd"][:], t1m[:], op.mult)
            nc.vector.tensor_tensor(dnum[:], dnum[:], tmp[:], op.add)
            nc.vector.tensor_tensor(tmp[:], planes["d0g"][:], m1t[:], op.mult)
            nc.vector.tensor_tensor(dnum[:], dnum[:], tmp[:], op.add)

            rden = pool.tile([D, fd], f32, tag="rden")
            nc.vector.reciprocal_approx_accurate(rden[:], den[:], tmp[:])
            nc.vector.tensor_tensor(num[:], num[:], rden[:], op.mult)
            outt = pool.tile([D, fd], f32, tag="outt")
            nc.vector.tensor_tensor(outt[:], num[:], planes["ich"][:], op.add)

            lnden = pool.tile([D, fd], f32, tag="lnden")
            nc.scalar.activation(lnden[:], den[:], AF.Ln)
            lndn = pool.tile([D, fd], f32, tag="lndn")
            nc.scalar.activation(lndn[:], dnum[:], AF.Ln)
            lad = pool.tile([D, fd], f32, tag="lad")
            nc.vector.scalar_tensor_tensor(lad[:], lnden[:], -2.0, lndn[:],
                                           op.mult, op.add)

            inside = pool.tile([D, fd], f32, tag="inside")
            nc.vector.tensor_tensor(inside[:], xT[:], xc[:], op.is_equal)
            nc.vector.tensor_tensor(lad[:], lad[:], inside[:], op.mult)
            insideu = pool.tile([D, fd], mybir.dt.uint8, tag="insideu")
            nc.vector.tensor_tensor(insideu[:], xT[:], xc[:], op.is_equal)
            outf = iopool.tile([D, fd], f32, tag="outf")
            nc.vector.tensor_copy(outf[:], xT[:])
            nc.vector.copy_predicated(outf[:], insideu[:], outt[:])

            pld = ppool.tile([1, fd], f32, tag="pld")
            for j in range(0, fd, 512):
                je = min(j + 512, fd)
                nc.tensor.matmul(pld[:, j:je], ones[:], lad[:, j:je],
                                 start=True, stop=True)
            lds = iopool.tile([1, fd], f32, tag="lds")
            nc.scalar.copy(lds[:], pld[:])

            nc.sync.dma_start(u_out[cs:cs + fd, :].rearrange("a b -> b a"), outf[:])
            nc.sync.dma_start(ld_out[0:1, cs:cs + fd], lds[:])

    nc.compile()
    return nc


_NC_CACHE = {}


def _get_nc(b_shard, fd):
    key = (b_shard, fd)
    if key not in _NC_CACHE:
        _NC_CACHE[key] = build_nc(b_shard, fd)
    return _NC_CACHE[key]


def kernel(x, unnormalized_widths, unnormalized_heights, unnormalized_derivatives):
    from concourse import bass_utils

    x = np.ascontiguousarray(np.asarray(x, np.float32))
    tab = _make_tab(unnormalized_widths, unnormalized_heights,
                    unnormalized_derivatives)

    nc = _get_nc(B_SHARD, 1024)
    shards = x.reshape(N_CORES, B_SHARD, D)
    in_maps = [{"x": shards[i], "tab": tab} for i in range(N_CORES)]
    res = bass_utils.run_bass_kernel_spmd(nc, in_maps, core_ids=list(range(N_CORES)))
    u = np.concatenate([r["u"] for r in res.results], axis=0)
    ld = np.concatenate([r["ld"].reshape(-1) for r in res.results], axis=0)
    return u, ld


# revision 12
# speedup vs baseline: 1.2339x; 1.2339x over previous
"""Trainium2 Bass kernel for nn_ComponentWiseSpline (rational-quadratic spline fwd).

Self-contained: hardcodes shapes B=65536, D=128, K=8, 8-core batch-data-parallel.

Algorithm (d-on-partitions layout, per core shard of B/8 rows):
  Host (numpy, f64->f32): spline tables -> 9 per-(d,bin) "plane" tables,
  each encoded as base + 7 telescoped deltas over step masks
  m_k = (xc >= knot_k + eps).  Device:
    xc   = clip(x, -3, 3)
    m_k  = (xc >= thr_k)                        k=1..7
    P_q  = base_q + sum_k m_k * delta_qk        (9 gathered planes)
    u    = xc - P_c;  th = u * P_rw;  th2 = th^2;  t1m = th - th2
    num* = P_ih*th2 + P_P1*t1m                  (= num/delta)
    den* = 1 + P_Rs*t1m                         (= den/delta)
    dnum'= P_d1g*th2 + P_dd*t1m + P_d0g*(th-1)^2 (= dnum/delta^2, positive form)
    out  = P_ich + num*/den*;   lad = ln(dnum') - 2 ln(den*)
    identity outside |x|<=3;  log_detJ = ones^T @ lad (PE partition-reduce)
"""
import os
import sys

for _p in ("/opt/trn_rl_repo", os.path.expanduser("~/.axon_site/_ro/trn_rl_repo")):
    if os.path.isdir(_p) and _p not in sys.path:
        sys.path.insert(0, _p)

import numpy as np

B, D, K = 65536, 128, 8
N_CORES = 8
B_SHARD = B // N_CORES
BOUND = 3.0
MIN_BIN_W = 1e-3
MIN_BIN_H = 1e-3
MIN_DERIV = 1e-3
EPS = 1e-6

PLANES = ["c", "rw", "ih", "P1", "Rs", "d1g", "dd", "d0g", "ich"]
NTAB = 7 + 8 * len(PLANES)  # 7 thresholds + (base + 7 deltas) per plane


# --------------------------------------------------------------------------- #
# Host-side table construction (numpy, f64 internally)
# --------------------------------------------------------------------------- #
def _make_tab(uw, uh, ud):
    uw = np.asarray(uw, np.float64)
    uh = np.asarray(uh, np.float64)
    ud = np.asarray(ud, np.float64)

    def softmax(v):
        e = np.exp(v - v.max(axis=-1, keepdims=True))
        return e / e.sum(axis=-1, keepdims=True)

    def softplus(v):
        return np.log1p(np.exp(-np.abs(v))) + np.maximum(v, 0)

    widths = MIN_BIN_W + (1.0 - MIN_BIN_W * K) * softmax(uw)
    heights = MIN_BIN_H + (1.0 - MIN_BIN_H * K) * softmax(uh)
    derivs = np.pad(MIN_DERIV + softplus(ud), ((0, 0), (1, 1)),
                    constant_values=1.0 - MIN_DERIV)  # [D, K+1]

    def knots(lengths):
        kn = np.cumsum(lengths, axis=-1)
        kn = np.pad(kn, ((0, 0), (1, 0)))
        kn = 2 * BOUND * kn - BOUND
        kn[:, 0] = -BOUND
        kn[:, -1] = BOUND
        return kn[:, 1:] - kn[:, :-1], kn

    w_, cw = knots(widths)
    h_, ch = knots(heights)
    delta = h_ / w_
    d0 = derivs[:, :K]
    d1 = derivs[:, 1:]

    tabs = {
        "c": cw[:, :K],
        "rw": 1.0 / w_,
        "ih": h_,
        "P1": h_ * d0 / delta,
        "Rs": (d0 + d1 - 2 * delta) / delta,
        "d1g": d1,
        "dd": 2 * delta,
        "d0g": d0,
        "ich": ch[:, :K],
    }

    tab = np.zeros((D, NTAB), np.float32)
    tab[:, 0:7] = (cw[:, 1:K] + EPS).astype(np.float32)
    for q, name in enumerate(PLANES):
        t = tabs[name].astype(np.float32).astype(np.float64)
        base = 7 + 8 * q
        tab[:, base] = t[:, 0].astype(np.float32)
        tab[:, base + 1:base + 8] = (t[:, 1:] - t[:, :-1]).astype(np.float32)
    return tab


# --------------------------------------------------------------------------- #
# Device kernel
# --------------------------------------------------------------------------- #
def build_nc(b_shard=B_SHARD, fd=512, repeat=1):
    import concourse.bacc as bacc
    import concourse.mybir as mybir
    from concourse.tile import TileContext
    from contextlib import ExitStack

    f32 = mybir.dt.float32
    op = mybir.AluOpType
    AF = mybir.ActivationFunctionType

    assert b_shard % fd == 0
    nchunks = b_shard // fd

    nc = bacc.Bacc("TRN2", target_bir_lowering=False, debug=False)
    x = nc.dram_tensor("x", [D, b_shard], f32, kind="ExternalInput")
    tabd = nc.dram_tensor("tab", [D, NTAB], f32, kind="ExternalInput")
    u_out = nc.dram_tensor("u", [D, b_shard], f32, kind="ExternalOutput")
    ld_out = nc.dram_tensor("ld", [1, b_shard], f32, kind="ExternalOutput")

    def col(tile, j):  # [128, 1] per-partition scalar view
        return tile[:, j:j + 1]

    with TileContext(nc) as tc, ExitStack() as ctx:
        cpool = ctx.enter_context(tc.tile_pool(name="const", bufs=1))
        iopool = ctx.enter_context(tc.tile_pool(name="io", bufs=1))
        pool = ctx.enter_context(tc.tile_pool(name="work", bufs=1))
        ppool = ctx.enter_context(tc.tile_pool(name="psum", bufs=2, space="PSUM"))

        tab = cpool.tile([D, NTAB], f32, tag="tab")
        nc.sync.dma_start(tab[:], tabd[:])
        ones = cpool.tile([D, 1], f32, tag="ones")
        nc.vector.memset(ones[:], 1.0)

        for ci in range(nchunks * repeat):
            cs = (ci % nchunks) * fd
            xT = iopool.tile([D, fd], f32, tag="xT")
            nc.sync.dma_start(xT[:], x[:, cs:cs + fd])

            xc = pool.tile([D, fd], f32, tag="xc")
            nc.vector.tensor_scalar(xc[:], xT[:], -BOUND, BOUND, op.max, op.min)

            masks = []
            for k in range(1, 8):
                m = pool.tile([D, fd], f32, tag=f"m{k}")
                nc.vector.tensor_scalar(m[:], xc[:], col(tab, k - 1), None, op.is_ge)
                masks.append(m)

            planes = {}
            for q, name in enumerate(PLANES):
                base = 7 + 8 * q
                p = pool.tile([D, fd], f32, tag=f"p_{name}")
                nc.vector.tensor_scalar(p[:], masks[0][:], col(tab, base + 1),
                                        col(tab, base), op.mult, op.add)
                for k in range(2, 8):
                    nc.vector.scalar_tensor_tensor(p[:], masks[k - 1][:],
                                                   col(tab, base + k), p[:],
                                                   op.mult, op.add)
                planes[name] = p

            t1 = pool.tile([D, fd], f32, tag="m1")   # u, then th
            nc.vector.tensor_tensor(t1[:], xc[:], planes["c"][:], op.subtract)
            nc.vector.tensor_tensor(t1[:], t1[:], planes["rw"][:], op.mult)  # th
            th2 = pool.tile([D, fd], f32, tag="m2")
            nc.vector.tensor_tensor(th2[:], t1[:], t1[:], op.mult)
            t1m = pool.tile([D, fd], f32, tag="m3")
            nc.vector.tensor_tensor(t1m[:], t1[:], th2[:], op.subtract)
            m1t = pool.tile([D, fd], f32, tag="m4")  # th-1, then (th-1)^2
            nc.vector.tensor_scalar(m1t[:], t1[:], 1.0, None, op.subtract)
            nc.vector.tensor_tensor(m1t[:], m1t[:], m1t[:], op.mult)

            num = pool.tile([D, fd], f32, tag="m5")
            tmp = pool.tile([D, fd], f32, tag="m6")
            nc.vector.tensor_tensor(num[:], planes["ih"][:], th2[:], op.mult)
            nc.vector.tensor_tensor(tmp[:], planes["P1"][:], t1m[:], op.mult)
            nc.vector.tensor_tensor(num[:], num[:], tmp[:], op.add)

            den = pool.tile([D, fd], f32, tag="m7")
            nc.vector.tensor_tensor(den[:], planes["Rs"][:], t1m[:], op.mult)
            nc.vector.tensor_scalar(den[:], den[:], 1.0, None, op.add)

            dnum = pool.tile([D, fd], f32, tag="p_c")
            nc.vector.tensor_tensor(dnum[:], planes["d1g"][:], th2[:], op.mult)
            nc.vector.tensor_tensor(tmp[:], planes["dd"][:], t1m[:], op.mult)
            nc.vector.tensor_tensor(dnum[:], dnum[:], tmp[:], op.add)
            nc.vector.tensor_tensor(tmp[:], planes["d0g"][:], m1t[:], op.mult)
            nc.vector.tensor_tensor(dnum[:], dnum[:], tmp[:], op.add)

            rden = pool.tile([D, fd], f32, tag="p_rw")
            nc.vector.reciprocal_approx_accurate(rden[:], den[:], tmp[:])
            nc.vector.tensor_tensor(num[:], num[:], rden[:], op.mult)
            outt = pool.tile([D, fd], f32, tag="p_ih")
            nc.vector.tensor_tensor(outt[:], num[:], planes["ich"][:], op.add)

            lnden = pool.tile([D, fd], f32, tag="p_P1")
            nc.scalar.activation(lnden[:], den[:], AF.Ln)
            lndn = pool.tile([D, fd], f32, tag="p_Rs")
            nc.scalar.activation(lndn[:], dnum[:], AF.Ln)
            lad = pool.tile([D, fd], f32, tag="p_d1g")
            nc.vector.scalar_tensor_tensor(lad[:], lnden[:], -2.0, lndn[:],
                                           op.mult, op.add)

            inside = pool.tile([D, fd], f32, tag="p_dd")
            nc.vector.tensor_tensor(inside[:], xT[:], xc[:], op.is_equal)
            nc.vector.tensor_tensor(lad[:], lad[:], inside[:], op.mult)
            insideu = pool.tile([D, fd], mybir.dt.uint8, tag="p_d0g")
            nc.vector.tensor_tensor(insideu[:], xT[:], xc[:], op.is_equal)
            outf = iopool.tile([D, fd], f32, tag="outf")
            nc.vector.tensor_copy(outf[:], xT[:])
            nc.vector.copy_predicated(outf[:], insideu[:], outt[:])

            pld = ppool.tile([1, fd], f32, tag="pld")
            for j in range(0, fd, 512):
                je = min(j + 512, fd)
                nc.tensor.matmul(pld[:, j:je], ones[:], lad[:, j:je],
                                 start=True, stop=True)
            lds = iopool.tile([1, fd], f32, tag="lds")
            nc.scalar.copy(lds[:], pld[:])

            nc.sync.dma_start(u_out[:, cs:cs + fd], outf[:])
            nc.sync.dma_start(ld_out[0:1, cs:cs + fd], lds[:])

    nc.compile()
    return nc


_NC_CACHE = {}


def _get_nc(b_shard, fd):
    key = (b_shard, fd)
    if key not in _NC_CACHE:
        _NC_CACHE[key] = build_nc(b_shard, fd)
    return _NC_CACHE[key]


def kernel(x, unnormalized_widths, unnormalized_heights, unnormalized_derivatives):
    from concourse import bass_utils

    x = np.ascontiguousarray(np.asarray(x, np.float32))
    tab = _make_tab(unnormalized_widths, unnormalized_heights,
                    unnormalized_derivatives)

    nc = _get_nc(B_SHARD, 2048)
    xT = np.ascontiguousarray(x.T)  # [D, B]
    in_maps = [{"x": np.ascontiguousarray(xT[:, i * B_SHARD:(i + 1) * B_SHARD]),
                "tab": tab} for i in range(N_CORES)]
    res = bass_utils.run_bass_kernel_spmd(nc, in_maps, core_ids=list(range(N_CORES)))
    u = np.concatenate([r["u"].T for r in res.results], axis=0)
    ld = np.concatenate([r["ld"].reshape(-1) for r in res.results], axis=0)
    return np.ascontiguousarray(u), ld
